# revision 5
# baseline (speedup 1.0000x reference)
# Trainium2 Bass kernel for nn_CalcDeformation (scatter 1024 betas onto a
# regular 32x32 stride-8 grid in a 256x256 image, depthwise-conv with a
# shared 31x31 kernel, 2 channels, batch 128 -> output [128, 65536, 2]).
#
# Because the scatter centers form a regular stride-8 grid, scatter+conv is
# a transposed convolution. Writing output rows R = 8*qr + pr, each output
# row only sees 4 consecutive control-grid rows gr = qr + e:
#   out[b,ch,R,C] = sum_{gr,gc} beta[b,gr,gc,ch] * K[rk-R+15, ck-C+15],
#   (rk,ck) = (8gr+4, 8gc+4);  kernel row = 8e-pr+19, col = 8gc-C+19,
#   e in {-2..1} for pr 0..3 (group g=0), e in {-1..2} for pr 4..7 (g=1).
#
# This becomes one 128-contraction matmul per (batch-chunk, group, channel,
# row-phase pair):   out[(b,qr), (pr,C)] = L^T @ W   with
#   L[(jj,gc'), (b,qr)] = beta[b, qr+e_base(g)+jj, 31-gc', ch]      (lhsT)
#   W[(jj,gc'), (pr,C)] = K[8jj+3-pr+4g, 8*(31-gc')-C+19]   (0 if invalid)
#
# Everything on-device is bf16 except the PSUM accumulation (f32): inputs
# L/W are bf16 (halves load traffic and PE power -> less utilization
# throttling than fp32r), and the output is stored to DRAM as bf16 and
# widened to f32 on the host (the problem tolerance is 2e-2 relative to
# max|out|; bf16 rounding contributes ~4e-3). This halves the dominant
# store traffic: total DMA is ~5.2 MB/core instead of ~10.5 MB/core.
#
# Sharding: pure batch data parallel, 16 batches per core on 8 cores.
# Host-side prep is pure indexing (permutation/replication of inputs into
# the lhsT/W layouts); all arithmetic runs on device.
import os

import ml_dtypes
import numpy as np

import concourse.bass as bass
import concourse.bacc as bacc
import concourse.mybir as mybir
import concourse.tile as tile
from concourse.bass_utils import run_bass_kernel_spmd

F32 = mybir.dt.float32
BF16 = mybir.dt.bfloat16
NP_BF16 = ml_dtypes.bfloat16

N_CORES = 8
BATCH = 128
B_L = BATCH // N_CORES
KS = 31
IMG = 256
N_OUT = B_L * IMG * IMG * 2


def _ap(t, off, pat):
    return bass.AP(tensor=t.ap().tensor, offset=off, ap=[list(p) for p in pat])


def _host_prepare_w(kern):
    """kern [31,31] -> w2 [128, 2048] bf16:
    w2[jj*32+gc', (g, pr, C)] = K[8jj+3-pr+4g, 8*(31-gc')-C+19] or 0."""
    kp = np.zeros((KS, 504), np.float32)
    kp[:, 237:268] = kern[:, ::-1]
    swv = np.lib.stride_tricks.sliding_window_view(kp, 256, axis=1)
    w2 = np.zeros((4, 32, 2, 4, 256), np.float32)  # [jj, gc', g, pr, C]
    cols = 8 * np.arange(32)
    for g in range(2):
        for jj in range(4):
            for pr in range(4):
                kr = 8 * jj + 3 - pr + 4 * g
                if 0 <= kr <= 30:
                    w2[jj, :, g, pr, :] = swv[kr, cols, :]
    return w2.reshape(128, 2048).astype(NP_BF16)


def _host_prepare_l(betas_core):
    """betas_core [B_L,1024,2] (k = gr*32+gc) -> l5 [128, 2048] bf16:
    l5[jj*32+gc', (chunk, t=(ch*2+g), b_local, qr)] =
        beta[chunk*4+b_local, qr+e_base(g)+jj, 31-gc', ch] (0 out of range)."""
    bg = betas_core.reshape(B_L, 32, 32, 2)
    l4 = np.zeros((4, 32, 4, B_L, 32), np.float32)  # [jj, gc', t, b, qr]
    for ch in range(2):
        for g in range(2):
            e_base = -2 + g
            t = ch * 2 + g
            for jj in range(4):
                e = e_base + jj
                lo = max(0, -e)
                cnt = 32 - abs(e)
                l4[jj, :, t, :, lo:lo + cnt] = (
                    bg[:, lo + e:lo + e + cnt, ::-1, ch].transpose(2, 0, 1)
                )
    l5 = l4.reshape(128, 4, 4, 4, 32).transpose(0, 2, 1, 3, 4)
    return np.ascontiguousarray(l5).reshape(128, 2048).astype(NP_BF16)


def _build_nc():
    nc = bacc.Bacc("TRN2", target_bir_lowering=False, debug=False,
                   num_devices=N_CORES)
    l4d = nc.dram_tensor("l4", [128 * 2048], BF16, kind="ExternalInput")
    w2d = nc.dram_tensor("w2", [128 * 2048], BF16, kind="ExternalInput")
    out = nc.dram_tensor("out", [N_OUT], BF16, kind="ExternalOutput")

    with tile.TileContext(nc) as tc:
        with (
            tc.tile_pool(name="wp", bufs=1) as wp,
            tc.tile_pool(name="lp", bufs=1) as lp,
            tc.tile_pool(name="sp", bufs=8) as sp,
            tc.tile_pool(name="pp", bufs=4, space="PSUM") as pp,
        ):
            W = wp.tile([128, 2048], BF16, tag="w")
            L = lp.tile([128, 2048], BF16, tag="l")

            def load_w(c0, c1):
                nc.sync.dma_start(
                    out=W[:, c0:c1],
                    in_=_ap(w2d, c0, [[2048, 128], [1, c1 - c0]]))

            def load_l(chunk):
                off = chunk * 512
                nc.scalar.dma_start(
                    out=L[:, off:off + 512],
                    in_=_ap(l4d, off, [[2048, 128], [1, 512]]))

            def piece(chunk, g, prpair, idx):
                p0 = 2 * prpair
                S = sp.tile([128, 1024], BF16, tag="stage")
                # One 2-bank PSUM tile per piece: cols = (ch, pr, C).
                P = pp.tile([128, 1024], F32, tag="psum")
                for ch in range(2):
                    t = ch * 2 + g
                    nc.tensor.matmul(
                        P[:, ch * 512:(ch + 1) * 512],
                        lhsT=L[:, chunk * 512 + t * 128:
                               chunk * 512 + (t + 1) * 128],
                        rhs=W[:, g * 1024 + p0 * 256:
                              g * 1024 + p0 * 256 + 512],
                        start=True, stop=True,
                    )
                # Single interleaving copy (f32 PSUM -> bf16 SBUF),
                # alternating between the two otherwise-idle copy engines.
                dstv = S[:].rearrange("p (pr c two) -> p pr c two",
                                      pr=2, two=2)
                inv = P[:].rearrange("p (two pr c) -> p pr c two",
                                     two=2, pr=2)
                if idx % 2 == 0:
                    nc.vector.tensor_copy(dstv, inv)
                else:
                    nc.scalar.copy(dstv, inv)
                deng = nc.sync if idx % 2 == 0 else nc.scalar
                deng.dma_start(
                    out=_ap(out,
                            chunk * 4 * 131072 + g * 2048 + p0 * 512,
                            [[131072, 4], [4096, 32], [1, 1024]]),
                    in_=S[:],
                )

            load_l(0)
            load_w(0, 512)
            load_w(512, 1024)
            idx = 0
            piece(0, 0, 0, idx); idx += 1
            load_w(1024, 2048)
            piece(0, 0, 1, idx); idx += 1
            load_l(1)
            piece(0, 1, 0, idx); idx += 1
            piece(0, 1, 1, idx); idx += 1
            load_l(2)
            piece(1, 0, 0, idx); idx += 1
            piece(1, 0, 1, idx); idx += 1
            load_l(3)
            piece(1, 1, 0, idx); idx += 1
            piece(1, 1, 1, idx); idx += 1
            for chunk in range(2, 4):
                for g in range(2):
                    for prpair in range(2):
                        piece(chunk, g, prpair, idx)
                        idx += 1
    nc.compile()
    return nc


_NC_CACHE = None


def _get_nc():
    global _NC_CACHE
    if _NC_CACHE is None:
        _NC_CACHE = _build_nc()
    return _NC_CACHE


def _grid_permute(betas, g_centers):
    """Reorder betas so that k = gr*32 + gc (row-major regular grid)."""
    rows = g_centers[:, 0].astype(np.int64)
    cols = g_centers[:, 1].astype(np.int64)
    gr, gc = (rows - 4) // 8, (cols - 4) // 8
    ok = (np.array_equal(rows, gr * 8 + 4) and np.array_equal(cols, gc * 8 + 4)
          and gr.min() >= 0 and gr.max() < 32
          and gc.min() >= 0 and gc.max() < 32)
    if not ok:
        raise NotImplementedError("g_centers is not the regular 32x32 grid")
    gidx = gr * 32 + gc
    if len(np.unique(gidx)) != 1024:
        raise NotImplementedError("duplicate g_centers")
    bg = np.empty_like(betas)
    bg[:, gidx, :] = betas
    return bg


LAST_RESULTS = None  # BassKernelResults of the most recent run (for test.py)


def kernel(betas, kernel, g_centers):
    betas = np.ascontiguousarray(np.asarray(betas, dtype=np.float32))
    kern = np.asarray(kernel, dtype=np.float32)
    g_centers = np.asarray(g_centers)
    assert betas.shape == (BATCH, 1024, 2) and kern.shape == (KS, KS)

    bg = _grid_permute(betas, g_centers)
    w2 = _host_prepare_w(kern).reshape(-1)
    in_maps = [
        {"l4": _host_prepare_l(bg[c * B_L:(c + 1) * B_L]).reshape(-1),
         "w2": w2}
        for c in range(N_CORES)
    ]

    nc = _get_nc()
    trace = os.environ.get("DEFORM_TRACE", "") == "1"
    res = run_bass_kernel_spmd(nc, in_maps, core_ids=list(range(N_CORES)),
                               trace=trace)
    global LAST_RESULTS
    LAST_RESULTS = res

    out = np.empty((BATCH, IMG * IMG, 2), np.float32)
    for c in range(N_CORES):
        out[c * B_L:(c + 1) * B_L] = np.asarray(
            res.results[c]["out"]).astype(np.float32).reshape(
            B_L, IMG * IMG, 2)
    return out
